# revision 1
# baseline (speedup 1.0000x reference)
"""Trainium2 Bass kernel: entmax-1.5 along the last dim of x[8,16,1024,1024] f32.

Takes the FULL unsharded input, shards rows data-parallel across 8 NeuronCores
(pure rowwise op, no communication), runs a Bass/Tile kernel per core via
run_bass_kernel_spmd, and gathers the full output.

Per-row algorithm (d=1024, fp32), mathematically identical to the sorted
closed form of Peters et al. 2019 but with no sorting:
  u = x/2 - max(x/2); y = relu(u - tau)^2 with tau s.t. sum_row(y) = 1.
  tau is the fixed point of the support-set closed form:
    S(t) = {i: x_i > t};  k=|S|, s1=sum_S v, s2=sum_S v^2 with v=(x-t)/2
    theta = (s1 - sqrt(max(s1^2 - k*(s2-1), 0)))/k;  t <- t + 2*theta
  Seeded from the k=8 closed form over the row's top-8 values, 3 updates
  reach the exact fixed point for every row of this input distribution.
  Once S(t) is the true support, the update IS the reference closed form,
  so the result matches the sort-based reference to fp32 accuracy.

HW notes (all verified by probe kernels on the device):
  - ACT Sqrt is a ~17-bit approximation (rel err 6.9e-6) -> one Newton
    step s' = 0.5*(s + d*recip(s)) using the exact DVE reciprocal.
  - ACT/DVE accum_out accumulate fp32 strictly left-to-right.
  - Pool (GPSIMD) cannot run accum_out or scalar_tensor_tensor; it can run
    2-op tensor_scalar / tensor_tensor.
  - tensor_scalar with accum_out: elementwise out is only (in0 op0 s1);
    op1 is the reduction op.

Engine schedule per [128,1024] row-tile (ns, est):
  DVE : top8 max 1067 | sum-r x3 1599 | relu(it3) 533 | count x2 1066
  ACT : Square+accum x3 2559 | final Square 853 | Sign count(it3) 853
  Pool: relu(it1) 1422 | relu(it2) 1422 | final relu 1422
"""

import sys

sys.path.insert(0, "/opt/trn_rl_repo")
sys.path.insert(0, "/opt/trn_rl_repo/concourse")

from contextlib import ExitStack

import numpy as np

D = 1024
P = 128
N_CORES = 8


def build_program(n_rows, group_tiles=16, dma_batch=4, debug=False,
                  rp_bufs=6, tr_bufs=3, smp_bufs=3, xp_mult=2, yp_bufs=4,
                  upd_eng='pool', ablate=(), relu_split=True):
    import concourse.bacc as bacc
    import concourse.tile as tile
    from concourse import mybir

    F32 = mybir.dt.float32
    ALU = mybir.AluOpType
    ACTF = mybir.ActivationFunctionType
    AX = mybir.AxisListType

    T = n_rows // P
    assert n_rows % P == 0 and T % group_tiles == 0
    assert group_tiles % dma_batch == 0
    n_groups = T // group_tiles
    G = group_tiles

    nc = bacc.Bacc(
        "TRN2", target_bir_lowering=False, debug=debug, enable_asserts=False
    )
    x = nc.dram_tensor("x", [n_rows, D], F32, kind="ExternalInput").ap()
    y = nc.dram_tensor("y", [n_rows, D], F32, kind="ExternalOutput").ap()

    with tile.TileContext(nc) as tc, ExitStack() as ctx:
        xp = ctx.enter_context(tc.tile_pool(name="xp", bufs=xp_mult * (G // dma_batch)))
        rp = ctx.enter_context(tc.tile_pool(name="rp", bufs=rp_bufs))
        dtr = ctx.enter_context(tc.tile_pool(name="dtr", bufs=tr_bufs))  # DVE trash
        atr = ctx.enter_context(tc.tile_pool(name="atr", bufs=tr_bufs))  # ACT trash
        yp = ctx.enter_context(tc.tile_pool(name="yp", bufs=yp_bufs))
        t8p = ctx.enter_context(tc.tile_pool(name="t8p", bufs=2))
        smp = ctx.enter_context(tc.tile_pool(name="smp", bufs=smp_bufs))

        def newton_sqrt(dn, tag):
            """sqrt(dn) to fp32 accuracy: ACT Sqrt approx + one Newton step.

            dn must be >= ~1e-30 (clamped) so recip stays finite."""
            r0 = smp.tile([P, G], F32, tag=tag + "r0")
            nc.scalar.activation(r0[:], dn[:], ACTF.Sqrt)
            rs = smp.tile([P, G], F32, tag=tag + "rs")
            nc.vector.reciprocal(rs[:], r0[:])
            t1 = smp.tile([P, G], F32, tag=tag + "t1")
            nc.gpsimd.tensor_mul(t1[:], dn[:], rs[:])
            t2 = smp.tile([P, G], F32, tag=tag + "t2")
            nc.gpsimd.tensor_add(t2[:], r0[:], t1[:])
            out = smp.tile([P, G], F32, tag=tag + "o")
            nc.gpsimd.tensor_scalar_mul(out[:], t2[:], 0.5)
            return out

        # --- stage emitters; emission order implements a 2-group software
        # pipeline so each engine's in-order stream always has ready work
        # while another group's update chain (serial latency) drains. ---
        state = {}

        def xv(g, j):
            return state[g]["xb"][j // dma_batch][:, j % dma_batch, :]

        def stage_load_seed(g):
            g_row0 = g * G * P
            xb = []
            for b in range(G // dma_batch):
                xt = xp.tile([P, dma_batch, D], F32, tag="x")
                r0 = g_row0 + b * dma_batch * P
                if "no_dma_in" not in ablate:
                    nc.sync.dma_start(
                        xt[:],
                        x[r0 : r0 + dma_batch * P, :].rearrange("(a p) m -> p a m", p=P),
                    )
                else:
                    nc.gpsimd.memset(xt[:, 0:1, 0:8], 0.5)
                xb.append(xt)
            state[g] = {"xb": xb}

            t8 = t8p.tile([P, G, 8], F32, tag="t8")
            for j in range(G):
                nc.vector.max(t8[:, j, :], xv(g, j))

            h8 = smp.tile([P, G, 8], F32, tag="h8")
            nc.gpsimd.tensor_scalar_mul(h8[:], t8[:], 0.5)
            u8 = smp.tile([P, G, 8], F32, tag="u8")
            nc.gpsimd.tensor_sub(
                u8[:], h8[:], h8[:, :, 0:1].to_broadcast([P, G, 8])
            )
            sq8 = smp.tile([P, G, 8], F32, tag="sq8")
            nc.gpsimd.tensor_mul(sq8[:], u8[:], u8[:])
            s1 = smp.tile([P, G], F32, tag="s1")
            nc.vector.tensor_reduce(s1[:], u8[:], axis=AX.X, op=ALU.add)
            s2 = smp.tile([P, G], F32, tag="s2")
            nc.vector.tensor_reduce(s2[:], sq8[:], axis=AX.X, op=ALU.add)
            # 4*disc = (2*s1)^2 - 32*(s2-1);  t0 = m + (2*s1 - sqrt(4disc))/8
            q1 = smp.tile([P, G], F32, tag="q1")
            nc.scalar.activation(q1[:], s1[:], ACTF.Square, scale=2.0)
            kd = smp.tile([P, G], F32, tag="kd")
            nc.gpsimd.tensor_scalar(kd[:], s2[:], -1.0, 32.0, op0=ALU.add, op1=ALU.mult)
            di = smp.tile([P, G], F32, tag="di")
            nc.gpsimd.tensor_sub(di[:], q1[:], kd[:])
            dn = smp.tile([P, G], F32, tag="dn")
            nc.gpsimd.tensor_scalar_max(dn[:], di[:], 1e-30)
            root = newton_sqrt(dn, "sd")
            s1d = smp.tile([P, G], F32, tag="s1d")
            nc.gpsimd.tensor_scalar_mul(s1d[:], s1[:], 2.0)
            num = smp.tile([P, G], F32, tag="num")
            nc.gpsimd.tensor_sub(num[:], s1d[:], root[:])
            th = smp.tile([P, G], F32, tag="th")
            nc.gpsimd.tensor_scalar_mul(th[:], num[:], 0.125)
            t_cur = smp.tile([P, G], F32, tag="t0")
            nc.gpsimd.tensor_add(t_cur[:], th[:], t8[:, :, 0])
            state[g]["t"] = t_cur

        def stage_iter(g, it):
            t_cur = state[g]["t"]
            A1 = smp.tile([P, G], F32, tag="A1")
            A2 = smp.tile([P, G], F32, tag="A2")
            if it == 0:
                # noisy-Newton: one DVE max+accum pass (s1 with benign
                # cancellation noise) + one biased ACT Square (exact s2).
                tm = smp.tile([P, G], F32, tag="tm")
                nc.gpsimd.tensor_scalar_mul(tm[:], t_cur[:], -1024.0)
                tb = smp.tile([P, G], F32, tag="tb")
                nc.gpsimd.tensor_scalar_mul(tb[:], t_cur[:], -0.5)
                for j in range(G):
                    t_col = t_cur[:, j : j + 1]
                    mo = rp.tile([P, D], F32, tag="r")
                    nc.vector.tensor_scalar(
                        mo[:], xv(g, j), t_col, tm[:, j : j + 1],
                        op0=ALU.max, op1=ALU.add,
                        accum_out=A1[:, j : j + 1],
                    )
                    sq = atr.tile([P, D], F32, tag="at")
                    nc.scalar.activation(
                        sq[:], mo[:], ACTF.Square, scale=0.5,
                        bias=tb[:, j : j + 1],
                        accum_out=A2[:, j : j + 1],
                    )
                # t += (A2 - 1)/s1, s1 = A1/2 (guarded)
                s1h = smp.tile([P, G], F32, tag="s1h")
                nc.gpsimd.tensor_scalar(
                    s1h[:], A1[:], 0.5, 1e-20, op0=ALU.mult, op1=ALU.max
                )
                rs = smp.tile([P, G], F32, tag="rsn")
                nc.vector.reciprocal(rs[:], s1h[:])
                am = smp.tile([P, G], F32, tag="am")
                nc.gpsimd.tensor_scalar(
                    am[:], A2[:], -1.0, None, op0=ALU.add
                )
                dtv = smp.tile([P, G], F32, tag="dtv")
                nc.gpsimd.tensor_mul(dtv[:], am[:], rs[:])
                t_new = smp.tile([P, G], F32, tag="tn")
                nc.gpsimd.tensor_add(t_new[:], dtv[:], t_cur[:])
                state[g]["t"] = t_new
                return
            A0 = smp.tile([P, G], F32, tag="A0")
            for j in range(G):
                t_col = t_cur[:, j : j + 1]
                r = rp.tile([P, D], F32, tag="r")
                # r = relu(x - t): mostly Pool, every 4th tile on DVE
                # (relu_split) to relieve the Pool pacing of CF iterations
                reng = nc.vector if (relu_split and j % 4 == 0) else nc.gpsimd
                reng.tensor_scalar(
                    r[:], xv(g, j), t_col, 0.0, op0=ALU.subtract, op1=ALU.max
                )
                # A1 = sum r (DVE 2x)
                tr = dtr.tile([P, D], F32, tag="dt")
                nc.vector.tensor_scalar(
                    tr[:], r[:], 0.0, None,
                    op0=ALU.add, op1=ALU.add,
                    accum_out=A1[:, j : j + 1],
                )
                # A2 = sum (r/2)^2 (ACT Square, exact)
                sq = atr.tile([P, D], F32, tag="at")
                nc.scalar.activation(
                    sq[:], r[:], ACTF.Square, scale=0.5,
                    accum_out=A2[:, j : j + 1],
                )
                # A0 = count(x > t) (DVE 2x)
                ct = dtr.tile([P, D], F32, tag="dt")
                nc.vector.tensor_scalar(
                    ct[:], xv(g, j), t_col, None,
                    op0=ALU.is_gt, op1=ALU.add,
                    accum_out=A0[:, j : j + 1],
                )
            # update: 4disc = A1^2 - 4k(A2-1); t += (A1 - sqrt(4disc))/k
            q1 = smp.tile([P, G], F32, tag="q1")
            nc.scalar.activation(q1[:], A1[:], ACTF.Square)
            m1 = smp.tile([P, G], F32, tag="m1")
            nc.gpsimd.tensor_scalar(
                m1[:], A2[:], -1.0, 4.0, op0=ALU.add, op1=ALU.mult
            )
            kd = smp.tile([P, G], F32, tag="kd")
            nc.gpsimd.tensor_mul(kd[:], m1[:], A0[:])
            di = smp.tile([P, G], F32, tag="di")
            nc.gpsimd.tensor_sub(di[:], q1[:], kd[:])
            dn = smp.tile([P, G], F32, tag="dn")
            nc.gpsimd.tensor_scalar_max(dn[:], di[:], 1e-30)
            root = newton_sqrt(dn, "it")
            num = smp.tile([P, G], F32, tag="num")
            nc.gpsimd.tensor_sub(num[:], A1[:], root[:])
            kg = smp.tile([P, G], F32, tag="kg")
            nc.gpsimd.tensor_scalar_max(kg[:], A0[:], 1.0)
            kinv = smp.tile([P, G], F32, tag="kinv")
            nc.vector.reciprocal(kinv[:], kg[:])
            pr = smp.tile([P, G], F32, tag="pr")
            nc.gpsimd.tensor_mul(pr[:], num[:], kinv[:])
            t_new = smp.tile([P, G], F32, tag="tn")
            nc.gpsimd.tensor_add(t_new[:], pr[:], t_cur[:])
            state[g]["t"] = t_new

        def stage_final(g):
            g_row0 = g * G * P
            t_cur = state[g]["t"]
            for j in range(G):
                t_col = t_cur[:, j : j + 1]
                r = rp.tile([P, D], F32, tag="r")
                nc.gpsimd.tensor_scalar(
                    r[:], xv(g, j), t_col, 0.0, op0=ALU.subtract, op1=ALU.max
                )
                yt = yp.tile([P, D], F32, tag="y")
                nc.scalar.activation(yt[:], r[:], ACTF.Square, scale=0.5)
                r0 = g_row0 + j * P
                if "no_dma_out" not in ablate:
                    nc.sync.dma_start(y[r0 : r0 + P, :], yt[:])

        # pair-pipelined emission
        for p0 in range(0, n_groups, 2):
            pair = [g for g in (p0, p0 + 1) if g < n_groups]
            for g in pair:
                stage_load_seed(g)
            for it in range(3):
                for g in pair:
                    stage_iter(g, it)
            for g in pair:
                stage_final(g)

    nc.compile()
    return nc


_PROGRAM = None
_PROGRAM_ROWS = None


def _get_program(rows_per_core):
    global _PROGRAM, _PROGRAM_ROWS
    if _PROGRAM is None or _PROGRAM_ROWS != rows_per_core:
        _PROGRAM = build_program(rows_per_core)
        _PROGRAM_ROWS = rows_per_core
    return _PROGRAM


def run_sharded(flat_x, trace=False):
    """flat_x: [n_rows, 1024] fp32. Returns (y, BassKernelResults)."""
    from concourse.bass_utils import run_bass_kernel_spmd

    n_rows = flat_x.shape[0]
    rows_per = n_rows // N_CORES
    assert rows_per * N_CORES == n_rows
    nc = _get_program(rows_per)
    in_maps = [
        {"x": np.ascontiguousarray(flat_x[i * rows_per : (i + 1) * rows_per])}
        for i in range(N_CORES)
    ]
    res = run_bass_kernel_spmd(nc, in_maps, list(range(N_CORES)), trace=trace)
    y = np.concatenate([res.results[i]["y"] for i in range(N_CORES)], axis=0)
    return y, res


def kernel(x):
    x = np.ascontiguousarray(np.asarray(x), dtype=np.float32)
    orig_shape = x.shape
    flat = x.reshape(-1, D)
    y, _ = run_sharded(flat)
    return y.reshape(orig_shape)



# revision 2
# speedup vs baseline: 7.2489x; 7.2489x over previous
"""Trainium2 Bass kernel: entmax-1.5 along the last dim of x[8,16,1024,1024] f32.

Takes the FULL unsharded input, shards rows data-parallel across 8 NeuronCores
(pure rowwise op, no communication), runs a Bass/Tile kernel per core via
run_bass_kernel_spmd, and gathers the full output.

Per-row algorithm (d=1024, fp32). Solves for tau* with
f(t) = sum_j relu((x_j - t)/2)^2 - 1 = 0 (f convex decreasing), then
y = relu((x - tau*)/2)^2.

  1. seed: t0 from the exact support-8 closed form over the row's top-8
     values (DVE max8): t0 = m + (s1 - sqrt(s1^2 - 8*(s2-4)))/8 with
     m = row max, s1/s2 = sum / sum-of-squares of (top8 - m).
  2. two Newton steps t <- t + (A2 - 1)/(A1/2), each needing only two
     full-D passes:
       DVE  tensor_scalar(max,add-accum):  mo = max(x,t), A1 = sum mo - 1024t
                                           (= sum relu(x-t), benign noise)
       ACT  Square(0.5*mo - 0.5*t)+accum:  A2 = sum relu((x-t)/2)^2  (exact)
     Newton from below is monotone (t0 <= t1 <= t2 <= tau*), quadratic.
  3. fused final: y = Square(0.5*mo@t1 - 0.5*t2). Since t1 <= t2 <= tau*,
     relu(x-t2) == relu(max(x,t1)-t2) except for x in (t1, t2] where the
     error is <= ((t2-t1)/2)^2 ~ 1e-3 * 1e-3 -- far below tolerance.

Accuracy vs the sorted reference (measured on the real input distribution in
an fp32-faithful numpy sim): max rel err 2.4e-3, vs the 2e-2 gate.

Per-tile engine budget (cost model, [128,1024] f32):
  DVE  max8 1127 + 2x mo/s1 594        = 2315 ns
  ACT  2x Square-accum 1133 + final 946 = 3212 ns
  Pool small t-update ops only          ~ 300 ns
  DMA  in 1754 + out 1754               = 3508 ns  <- roofline (memory-bound)
"""

import sys

sys.path.insert(0, "/opt/trn_rl_repo")
sys.path.insert(0, "/opt/trn_rl_repo/concourse")

from contextlib import ExitStack

import numpy as np

D = 1024
P = 128
N_CORES = 8


def build_program(n_rows, group_tiles=8, dma_batch=2, debug=False,
                  xp_bufs=8, mos_bufs=4, yp_bufs=3, n_newton=2, ablate=()):
    import concourse.bacc as bacc
    import concourse.tile as tile
    from concourse import mybir

    F32 = mybir.dt.float32
    ALU = mybir.AluOpType
    ACTF = mybir.ActivationFunctionType
    AX = mybir.AxisListType

    T = n_rows // P
    G = group_tiles
    assert n_rows % P == 0 and T % G == 0 and G % dma_batch == 0
    n_groups = T // G

    nc = bacc.Bacc(
        "TRN2", target_bir_lowering=False, debug=debug, enable_asserts=False
    )
    x = nc.dram_tensor("x", [n_rows, D], F32, kind="ExternalInput").ap()
    y = nc.dram_tensor("y", [n_rows, D], F32, kind="ExternalOutput").ap()

    with tile.TileContext(nc) as tc, ExitStack() as ctx:
        xp = ctx.enter_context(tc.tile_pool(name="xp", bufs=xp_bufs))
        mos = ctx.enter_context(tc.tile_pool(name="mos", bufs=mos_bufs))
        mol = ctx.enter_context(tc.tile_pool(name="mol", bufs=2 * G))
        atr = ctx.enter_context(tc.tile_pool(name="atr", bufs=3))
        yp = ctx.enter_context(tc.tile_pool(name="yp", bufs=yp_bufs))
        t8p = ctx.enter_context(tc.tile_pool(name="t8p", bufs=2))
        smp = ctx.enter_context(tc.tile_pool(name="smp", bufs=3))

        state = {}

        def xv(g, j):
            return state[g]["xb"][j // dma_batch][:, j % dma_batch, :]

        def stage_load_seed(g):
            g_row0 = g * G * P
            xb = []
            for b in range(G // dma_batch):
                xt = xp.tile([P, dma_batch, D], F32, tag="x")
                r0 = g_row0 + b * dma_batch * P
                if "no_dma_in" not in ablate:
                    nc.sync.dma_start(
                        xt[:],
                        x[r0 : r0 + dma_batch * P, :].rearrange("(a p) m -> p a m", p=P),
                    )
                else:
                    nc.gpsimd.memset(xt[:, 0:1, 0:8], 0.5)
                xb.append(xt)
            state[g] = {"xb": xb}

            # top-8 per row (descending); m = t8[:,:,0]
            t8 = t8p.tile([P, G, 8], F32, tag="t8")
            for j in range(G):
                nc.vector.max(t8[:, j, :], xv(g, j))

            # seed from support-8 closed form (x units)
            u8 = smp.tile([P, G, 8], F32, tag="u8")
            nc.gpsimd.tensor_sub(
                u8[:], t8[:], t8[:, :, 0:1].to_broadcast([P, G, 8])
            )
            sq8 = smp.tile([P, G, 8], F32, tag="sq8")
            nc.gpsimd.tensor_mul(sq8[:], u8[:], u8[:])
            s1 = smp.tile([P, G], F32, tag="s1")
            nc.vector.tensor_reduce(s1[:], u8[:], axis=AX.X, op=ALU.add)
            s2 = smp.tile([P, G], F32, tag="s2")
            nc.vector.tensor_reduce(s2[:], sq8[:], axis=AX.X, op=ALU.add)
            q1 = smp.tile([P, G], F32, tag="q1")
            nc.scalar.activation(q1[:], s1[:], ACTF.Square)
            kd = smp.tile([P, G], F32, tag="kd")
            nc.gpsimd.tensor_scalar(kd[:], s2[:], -4.0, 8.0, op0=ALU.add, op1=ALU.mult)
            di = smp.tile([P, G], F32, tag="di")
            nc.gpsimd.tensor_sub(di[:], q1[:], kd[:])
            dn = smp.tile([P, G], F32, tag="dn")
            nc.gpsimd.tensor_scalar_max(dn[:], di[:], 1e-30)
            root = smp.tile([P, G], F32, tag="root")
            nc.scalar.activation(root[:], dn[:], ACTF.Sqrt)
            num = smp.tile([P, G], F32, tag="num")
            nc.gpsimd.tensor_sub(num[:], s1[:], root[:])
            th = smp.tile([P, G], F32, tag="th")
            nc.gpsimd.tensor_scalar_mul(th[:], num[:], 0.125)
            t_cur = smp.tile([P, G], F32, tag="t0")
            nc.gpsimd.tensor_add(t_cur[:], th[:], t8[:, :, 0])
            state[g]["t"] = t_cur

        def stage_newton(g, it):
            t_cur = state[g]["t"]
            last = it == n_newton - 1
            A1 = smp.tile([P, G], F32, tag="A1")
            A2 = smp.tile([P, G], F32, tag="A2")
            tm = smp.tile([P, G], F32, tag="tm")
            nc.gpsimd.tensor_scalar_mul(tm[:], t_cur[:], -1024.0)
            tb = smp.tile([P, G], F32, tag="tb")
            nc.gpsimd.tensor_scalar_mul(tb[:], t_cur[:], -0.5)
            mo_tiles = []
            for j in range(G):
                t_col = t_cur[:, j : j + 1]
                mo = (mol if last else mos).tile([P, D], F32, tag="mo")
                nc.vector.tensor_scalar(
                    mo[:], xv(g, j), t_col, tm[:, j : j + 1],
                    op0=ALU.max, op1=ALU.add,
                    accum_out=A1[:, j : j + 1],
                )
                sq = atr.tile([P, D], F32, tag="at")
                nc.scalar.activation(
                    sq[:], mo[:], ACTF.Square, scale=0.5,
                    bias=tb[:, j : j + 1],
                    accum_out=A2[:, j : j + 1],
                )
                mo_tiles.append(mo)
            if last:
                state[g]["mo"] = mo_tiles
            # t += (A2 - 1) / max(A1/2, eps)
            s1h = smp.tile([P, G], F32, tag="s1h")
            nc.gpsimd.tensor_scalar(
                s1h[:], A1[:], 0.5, 1e-20, op0=ALU.mult, op1=ALU.max
            )
            rs = smp.tile([P, G], F32, tag="rs")
            nc.vector.reciprocal(rs[:], s1h[:])
            am = smp.tile([P, G], F32, tag="am")
            nc.gpsimd.tensor_scalar(am[:], A2[:], -1.0, None, op0=ALU.add)
            dtv = smp.tile([P, G], F32, tag="dtv")
            nc.gpsimd.tensor_mul(dtv[:], am[:], rs[:])
            t_new = smp.tile([P, G], F32, tag="tn")
            nc.gpsimd.tensor_add(t_new[:], dtv[:], t_cur[:])
            state[g]["t"] = t_new

        def stage_final(g):
            g_row0 = g * G * P
            t_cur = state[g]["t"]
            tb2 = smp.tile([P, G], F32, tag="tb2")
            nc.gpsimd.tensor_scalar_mul(tb2[:], t_cur[:], -0.5)
            yt = None
            for j in range(G):
                if j % dma_batch == 0:
                    yt = yp.tile([P, dma_batch, D], F32, tag="y")
                mo = state[g]["mo"][j]
                nc.scalar.activation(
                    yt[:, j % dma_batch, :], mo[:], ACTF.Square, scale=0.5,
                    bias=tb2[:, j : j + 1],
                )
                if (j + 1) % dma_batch == 0:
                    r0 = g_row0 + (j + 1 - dma_batch) * P
                    if "no_dma_out" not in ablate:
                        nc.sync.dma_start(
                            y[r0 : r0 + dma_batch * P, :].rearrange(
                                "(a p) m -> p a m", p=P
                            ),
                            yt[:],
                        )
            state[g]["mo"] = None

        # pair-pipelined emission: two groups interleaved so each engine's
        # in-order stream has ready work while the other group's serial
        # t-update chain drains.
        for p0 in range(0, n_groups, 2):
            pair = [g for g in (p0, p0 + 1) if g < n_groups]
            for g in pair:
                stage_load_seed(g)
            for it in range(n_newton):
                for g in pair:
                    stage_newton(g, it)
            for g in pair:
                stage_final(g)

    nc.compile()
    return nc


_PROGRAM = None
_PROGRAM_ROWS = None


def _get_program(rows_per_core):
    global _PROGRAM, _PROGRAM_ROWS
    if _PROGRAM is None or _PROGRAM_ROWS != rows_per_core:
        _PROGRAM = build_program(rows_per_core)
        _PROGRAM_ROWS = rows_per_core
    return _PROGRAM


def run_sharded(flat_x, trace=False):
    """flat_x: [n_rows, 1024] fp32. Returns (y, BassKernelResults)."""
    from concourse.bass_utils import run_bass_kernel_spmd

    n_rows = flat_x.shape[0]
    rows_per = n_rows // N_CORES
    assert rows_per * N_CORES == n_rows
    nc = _get_program(rows_per)
    in_maps = [
        {"x": np.ascontiguousarray(flat_x[i * rows_per : (i + 1) * rows_per])}
        for i in range(N_CORES)
    ]
    res = run_bass_kernel_spmd(nc, in_maps, list(range(N_CORES)), trace=trace)
    y = np.concatenate([res.results[i]["y"] for i in range(N_CORES)], axis=0)
    return y, res


def kernel(x):
    x = np.ascontiguousarray(np.asarray(x), dtype=np.float32)
    orig_shape = x.shape
    flat = x.reshape(-1, D)
    y, _ = run_sharded(flat)
    return y.reshape(orig_shape)


# revision 10
# speedup vs baseline: 10.6133x; 1.4641x over previous
"""Trainium2 Bass kernel: entmax-1.5 along the last dim of x[8,16,1024,1024] f32.

Takes the FULL unsharded input, shards rows data-parallel across 8 NeuronCores
(pure rowwise op, no communication), runs a Bass/Tile kernel per core via
run_bass_kernel_spmd, and gathers the full output.

Per-row algorithm (d=1024, fp32). Solves for tau* with
f(t) = sum_j relu((x_j - t)/2)^2 - 1 = 0 (f convex decreasing), then
y = relu((x - tau*)/2)^2.

  1. seed: t0 from the exact support-8 closed form over the row's top-8
     values (DVE max8): t0 = m + (s1 - sqrt(s1^2 - 8*(s2-4)))/8 with
     m = row max, s1/s2 = sum / sum-of-squares of (top8 - m).
  2. two Newton steps t <- t + (A2 - 1)/(A1/2), each needing only two
     full-D passes:
       DVE  tensor_scalar(max,add-accum):  mo = max(x,t), A1 = sum mo - 1024t
                                           (= sum relu(x-t), benign noise)
       ACT  Square(0.5*mo - 0.5*t)+accum:  A2 = sum relu((x-t)/2)^2  (exact)
     Newton from below is monotone (t0 <= t1 <= t2 <= tau*), quadratic.
  3. fused final: y = Square(0.5*mo@t1 - 0.5*t2). Since t1 <= t2 <= tau*,
     relu(x-t2) == relu(max(x,t1)-t2) except for x in (t1, t2] where the
     error is <= ((t2-t1)/2)^2 ~ 1e-3 * 1e-3 -- far below tolerance.

Accuracy vs the sorted reference (measured on the real input distribution in
an fp32-faithful numpy sim): max rel err 2.4e-3, vs the 2e-2 gate.

Per-tile engine budget (cost model, [128,1024] f32):
  DVE  max8 1127 + 2x mo/s1 594        = 2315 ns
  ACT  2x Square-accum 1133 + final 946 = 3212 ns
  Pool small t-update ops only          ~ 300 ns
  DMA  in 1754 + out 1754               = 3508 ns  <- roofline (memory-bound)
"""

import sys

sys.path.insert(0, "/opt/trn_rl_repo")
sys.path.insert(0, "/opt/trn_rl_repo/concourse")

from contextlib import ExitStack

import numpy as np

D = 1024
P = 128
N_CORES = 8


def build_program(n_rows, group_tiles=8, dma_batch=2, debug=False,
                  xp_bufs=8, mos_bufs=4, yp_bufs=3, n_newton=2,
                  pool_final_mod=0, repeats=1, ablate=()):
    import concourse.bacc as bacc
    import concourse.tile as tile
    from concourse import mybir

    F32 = mybir.dt.float32
    ALU = mybir.AluOpType
    ACTF = mybir.ActivationFunctionType
    AX = mybir.AxisListType

    T = n_rows // P
    G = group_tiles
    assert n_rows % P == 0 and T % G == 0 and G % dma_batch == 0
    n_groups = T // G

    nc = bacc.Bacc(
        "TRN2", target_bir_lowering=False, debug=debug, enable_asserts=False
    )
    x = nc.dram_tensor("x", [n_rows, D], F32, kind="ExternalInput").ap()
    y = nc.dram_tensor("y", [n_rows, D], F32, kind="ExternalOutput").ap()

    with tile.TileContext(nc) as tc, ExitStack() as ctx:
        xp = ctx.enter_context(tc.tile_pool(name="xp", bufs=xp_bufs))
        mos = ctx.enter_context(tc.tile_pool(name="mos", bufs=mos_bufs))
        mol = ctx.enter_context(tc.tile_pool(name="mol", bufs=2 * G + 2))
        atr = ctx.enter_context(tc.tile_pool(name="atr", bufs=3))
        zp = ctx.enter_context(tc.tile_pool(name="zp", bufs=2))
        yp = ctx.enter_context(tc.tile_pool(name="yp", bufs=yp_bufs))
        t8p = ctx.enter_context(tc.tile_pool(name="t8p", bufs=2))
        smp = ctx.enter_context(tc.tile_pool(name="smp", bufs=3))

        state = {}

        def xv(g, j):
            return state[g]["xb"][j // dma_batch][:, j % dma_batch, :]

        def stage_load_seed(g):
            g_row0 = g * G * P
            xb = []
            for b in range(G // dma_batch):
                xt = xp.tile([P, dma_batch, D], F32, tag="x")
                r0 = g_row0 + b * dma_batch * P
                if "no_dma_in" not in ablate:
                    nc.sync.dma_start(
                        xt[:],
                        x[r0 : r0 + dma_batch * P, :].rearrange("(a p) m -> p a m", p=P),
                    )
                else:
                    nc.gpsimd.memset(xt[:, 0:1, 0:8], 0.5)
                xb.append(xt)
            state[g] = {"xb": xb}

            # top-8 per row (descending); m = t8[:,:,0]
            t8 = t8p.tile([P, G, 8], F32, tag="t8")
            for j in range(G):
                nc.vector.max(t8[:, j, :], xv(g, j))

            # seed from support-8 closed form (x units)
            u8 = smp.tile([P, G, 8], F32, tag="u8")
            nc.gpsimd.tensor_sub(
                u8[:], t8[:], t8[:, :, 0:1].to_broadcast([P, G, 8])
            )
            sq8 = smp.tile([P, G, 8], F32, tag="sq8")
            nc.gpsimd.tensor_mul(sq8[:], u8[:], u8[:])
            s1 = smp.tile([P, G], F32, tag="s1")
            nc.vector.tensor_reduce(s1[:], u8[:], axis=AX.X, op=ALU.add)
            s2 = smp.tile([P, G], F32, tag="s2")
            nc.vector.tensor_reduce(s2[:], sq8[:], axis=AX.X, op=ALU.add)
            q1 = smp.tile([P, G], F32, tag="q1")
            nc.scalar.activation(q1[:], s1[:], ACTF.Square)
            kd = smp.tile([P, G], F32, tag="kd")
            nc.gpsimd.tensor_scalar(kd[:], s2[:], -4.0, 8.0, op0=ALU.add, op1=ALU.mult)
            di = smp.tile([P, G], F32, tag="di")
            nc.gpsimd.tensor_sub(di[:], q1[:], kd[:])
            dn = smp.tile([P, G], F32, tag="dn")
            nc.gpsimd.tensor_scalar_max(dn[:], di[:], 1e-30)
            root = smp.tile([P, G], F32, tag="root")
            nc.scalar.activation(root[:], dn[:], ACTF.Sqrt)
            num = smp.tile([P, G], F32, tag="num")
            nc.gpsimd.tensor_sub(num[:], s1[:], root[:])
            th = smp.tile([P, G], F32, tag="th")
            nc.gpsimd.tensor_scalar_mul(th[:], num[:], 0.125)
            t_cur = smp.tile([P, G], F32, tag="t0")
            nc.gpsimd.tensor_add(t_cur[:], th[:], t8[:, :, 0])
            state[g]["t"] = t_cur

        def stage_newton(g, it):
            t_cur = state[g]["t"]
            last = it == n_newton - 1
            A1 = smp.tile([P, G], F32, tag="A1")
            A2 = smp.tile([P, G], F32, tag="A2")
            tm = smp.tile([P, G], F32, tag="tm")
            nc.gpsimd.tensor_scalar_mul(tm[:], t_cur[:], -1024.0)
            tb = smp.tile([P, G], F32, tag="tb")
            nc.gpsimd.tensor_scalar_mul(tb[:], t_cur[:], -0.5)
            mo_tiles = []
            for j in range(G):
                t_col = t_cur[:, j : j + 1]
                mo = (mol if last else mos).tile([P, D], F32, tag="mo")
                nc.vector.tensor_scalar(
                    mo[:], xv(g, j), t_col, tm[:, j : j + 1],
                    op0=ALU.max, op1=ALU.add,
                    accum_out=A1[:, j : j + 1],
                )
                sq = atr.tile([P, D], F32, tag="at")
                nc.scalar.activation(
                    sq[:], mo[:], ACTF.Square, scale=0.5,
                    bias=tb[:, j : j + 1],
                    accum_out=A2[:, j : j + 1],
                )
                mo_tiles.append(mo)
            if last:
                state[g]["mo"] = mo_tiles
            # t += (A2 - 1) / max(A1/2, eps)
            s1h = smp.tile([P, G], F32, tag="s1h")
            nc.gpsimd.tensor_scalar(
                s1h[:], A1[:], 0.5, 1e-20, op0=ALU.mult, op1=ALU.max
            )
            rs = smp.tile([P, G], F32, tag="rs")
            nc.vector.reciprocal(rs[:], s1h[:])
            am = smp.tile([P, G], F32, tag="am")
            nc.gpsimd.tensor_scalar(am[:], A2[:], -1.0, None, op0=ALU.add)
            dtv = smp.tile([P, G], F32, tag="dtv")
            nc.gpsimd.tensor_mul(dtv[:], am[:], rs[:])
            t_new = smp.tile([P, G], F32, tag="tn")
            nc.gpsimd.tensor_add(t_new[:], dtv[:], t_cur[:])
            state[g]["t"] = t_new

        def stage_final(g):
            g_row0 = g * G * P
            t_cur = state[g]["t"]
            tb2 = smp.tile([P, G], F32, tag="tb2")
            nc.gpsimd.tensor_scalar_mul(tb2[:], t_cur[:], -0.5)
            nt2 = smp.tile([P, G], F32, tag="nt2")
            nc.gpsimd.tensor_scalar_mul(nt2[:], t_cur[:], -1.0)
            yt = None
            for j in range(G):
                if j % dma_batch == 0:
                    yt = yp.tile([P, dma_batch, D], F32, tag="y")
                mo = state[g]["mo"][j]
                if pool_final_mod and j % pool_final_mod == 0:
                    # Pool path: z = (mo + (-t2)) * 0.5; y = z*z (relieves
                    # ACT, which otherwise sits above the HBM roofline).
                    # Scalar operands follow the HW-proven (AP, const)
                    # slotting: scalar1 = per-partition AP, scalar2 = const.
                    zt = zp.tile([P, D], F32, tag="z")
                    nc.gpsimd.tensor_scalar(
                        zt[:], mo[:], nt2[:, j : j + 1], 0.5,
                        op0=ALU.add, op1=ALU.mult,
                    )
                    nc.gpsimd.tensor_mul(yt[:, j % dma_batch, :], zt[:], zt[:])
                else:
                    nc.scalar.activation(
                        yt[:, j % dma_batch, :], mo[:], ACTF.Square, scale=0.5,
                        bias=tb2[:, j : j + 1],
                    )
                if (j + 1) % dma_batch == 0:
                    r0 = g_row0 + (j + 1 - dma_batch) * P
                    if "no_dma_out" not in ablate:
                        nc.sync.dma_start(
                            y[r0 : r0 + dma_batch * P, :].rearrange(
                                "(a p) m -> p a m", p=P
                            ),
                            yt[:],
                        )
            state[g]["mo"] = None

        # pair-pipelined emission: two groups interleaved so each engine's
        # in-order stream has ready work while the other group's serial
        # t-update chain drains. repeats>1 re-emits the whole pass (same
        # input, same output) for steady-state throughput measurement by
        # R-differencing; every rep writes identical bytes to y.
        for _ in range(repeats):
            for p0 in range(0, n_groups, 2):
                pair = [g for g in (p0, p0 + 1) if g < n_groups]
                for g in pair:
                    stage_load_seed(g)
                for it in range(n_newton):
                    for g in pair:
                        stage_newton(g, it)
                for g in pair:
                    stage_final(g)

    nc.compile()
    return nc


_PROGRAM = None
_PROGRAM_ROWS = None


def _get_program(rows_per_core):
    global _PROGRAM, _PROGRAM_ROWS
    if _PROGRAM is None or _PROGRAM_ROWS != rows_per_core:
        _PROGRAM = build_program(rows_per_core)
        _PROGRAM_ROWS = rows_per_core
    return _PROGRAM


def run_sharded(flat_x, trace=False):
    """flat_x: [n_rows, 1024] fp32. Returns (y, BassKernelResults)."""
    from concourse.bass_utils import run_bass_kernel_spmd

    n_rows = flat_x.shape[0]
    rows_per = n_rows // N_CORES
    assert rows_per * N_CORES == n_rows
    nc = _get_program(rows_per)
    in_maps = [
        {"x": np.ascontiguousarray(flat_x[i * rows_per : (i + 1) * rows_per])}
        for i in range(N_CORES)
    ]
    res = run_bass_kernel_spmd(nc, in_maps, list(range(N_CORES)), trace=trace)
    y = np.concatenate([res.results[i]["y"] for i in range(N_CORES)], axis=0)
    return y, res


def kernel(x):
    x = np.ascontiguousarray(np.asarray(x), dtype=np.float32)
    orig_shape = x.shape
    flat = x.reshape(-1, D)
    y, _ = run_sharded(flat)
    return y.reshape(orig_shape)


# revision 14
# speedup vs baseline: 10.6286x; 1.0014x over previous
"""Trainium2 Bass kernel: entmax-1.5 along the last dim of x[8,16,1024,1024] f32.

Takes the FULL unsharded input, shards rows data-parallel across 8 NeuronCores
(pure rowwise op, no communication), runs a Bass/Tile kernel per core via
run_bass_kernel_spmd, and gathers the full output.

Per-row algorithm (d=1024, fp32). Solves for tau* with
f(t) = sum_j relu((x_j - t)/2)^2 - 1 = 0 (f convex decreasing), then
y = relu((x - tau*)/2)^2.

  1. seed: t0 from the exact support-8 closed form over the row's top-8
     values (DVE max8): t0 = m + (s1 - sqrt(s1^2 - 8*(s2-4)))/8 with
     m = row max, s1/s2 = sum / sum-of-squares of (top8 - m).
  2. two Newton steps t <- t + (A2 - 1)/(A1/2), each needing only two
     full-D passes:
       DVE  tensor_scalar(max,add-accum):  mo = max(x,t), A1 = sum mo - 1024t
                                           (= sum relu(x-t), benign noise)
       ACT  Square(0.5*mo - 0.5*t)+accum:  A2 = sum relu((x-t)/2)^2  (exact)
     Newton from below is monotone (t0 <= t1 <= t2 <= tau*), quadratic.
  3. fused final: y = Square(0.5*mo@t1 - 0.5*t2). Since t1 <= t2 <= tau*,
     relu(x-t2) == relu(max(x,t1)-t2) except for x in (t1, t2] where the
     error is <= ((t2-t1)/2)^2 ~ 1e-3 * 1e-3 -- far below tolerance.

Accuracy vs the sorted reference (measured on the real input distribution in
an fp32-faithful numpy sim): max rel err 2.4e-3, vs the 2e-2 gate.

Per-tile engine budget (cost model, [128,1024] f32):
  DVE  max8 1127 + 2x mo/s1 594        = 2315 ns
  ACT  2x Square-accum 1133 + final 946 = 3212 ns
  Pool small t-update ops only          ~ 300 ns
  DMA  in 1754 + out 1754               = 3508 ns  <- roofline (memory-bound)
"""

import sys

sys.path.insert(0, "/opt/trn_rl_repo")
sys.path.insert(0, "/opt/trn_rl_repo/concourse")

from contextlib import ExitStack

import numpy as np

D = 1024
P = 128
N_CORES = 8


def build_program(n_rows, group_tiles=8, dma_batch=2, debug=False,
                  xp_bufs=8, mos_bufs=4, yp_bufs=3, n_newton=2,
                  pool_final_mod=3, repeats=1, final_exact=False,
                  seed_ahead=False, smp_bufs=3, ablate=()):
    import concourse.bacc as bacc
    import concourse.tile as tile
    from concourse import mybir

    F32 = mybir.dt.float32
    ALU = mybir.AluOpType
    ACTF = mybir.ActivationFunctionType
    AX = mybir.AxisListType

    T = n_rows // P
    G = group_tiles
    assert n_rows % P == 0 and T % G == 0 and G % dma_batch == 0
    n_groups = T // G

    nc = bacc.Bacc(
        "TRN2", target_bir_lowering=False, debug=debug, enable_asserts=False
    )
    x = nc.dram_tensor("x", [n_rows, D], F32, kind="ExternalInput").ap()
    y = nc.dram_tensor("y", [n_rows, D], F32, kind="ExternalOutput").ap()

    with tile.TileContext(nc) as tc, ExitStack() as ctx:
        xp = ctx.enter_context(tc.tile_pool(name="xp", bufs=xp_bufs))
        mos = ctx.enter_context(tc.tile_pool(name="mos", bufs=mos_bufs))
        if not final_exact:
            mol = ctx.enter_context(tc.tile_pool(name="mol", bufs=2 * G + 2))
        atr = ctx.enter_context(tc.tile_pool(name="atr", bufs=3))
        zp = ctx.enter_context(tc.tile_pool(name="zp", bufs=2))
        yp = ctx.enter_context(tc.tile_pool(name="yp", bufs=yp_bufs))
        t8p = ctx.enter_context(tc.tile_pool(name="t8p", bufs=2))
        smp = ctx.enter_context(tc.tile_pool(name="smp", bufs=smp_bufs))

        state = {}

        def xv(g, j):
            return state[g]["xb"][j // dma_batch][:, j % dma_batch, :]

        def stage_load_seed(g):
            g_row0 = g * G * P
            xb = []
            for b in range(G // dma_batch):
                xt = xp.tile([P, dma_batch, D], F32, tag="x")
                r0 = g_row0 + b * dma_batch * P
                if "no_dma_in" not in ablate:
                    nc.sync.dma_start(
                        xt[:],
                        x[r0 : r0 + dma_batch * P, :].rearrange("(a p) m -> p a m", p=P),
                    )
                else:
                    nc.gpsimd.memset(xt[:, 0:1, 0:8], 0.5)
                xb.append(xt)
            state[g] = {"xb": xb}

            # top-8 per row (descending); m = t8[:,:,0]
            t8 = t8p.tile([P, G, 8], F32, tag="t8")
            for j in range(G):
                nc.vector.max(t8[:, j, :], xv(g, j))

            # seed from support-8 closed form (x units)
            u8 = smp.tile([P, G, 8], F32, tag="u8")
            nc.gpsimd.tensor_sub(
                u8[:], t8[:], t8[:, :, 0:1].to_broadcast([P, G, 8])
            )
            sq8 = smp.tile([P, G, 8], F32, tag="sq8")
            nc.gpsimd.tensor_mul(sq8[:], u8[:], u8[:])
            s1 = smp.tile([P, G], F32, tag="s1")
            nc.vector.tensor_reduce(s1[:], u8[:], axis=AX.X, op=ALU.add)
            s2 = smp.tile([P, G], F32, tag="s2")
            nc.vector.tensor_reduce(s2[:], sq8[:], axis=AX.X, op=ALU.add)
            q1 = smp.tile([P, G], F32, tag="q1")
            nc.scalar.activation(q1[:], s1[:], ACTF.Square)
            kd = smp.tile([P, G], F32, tag="kd")
            nc.gpsimd.tensor_scalar(kd[:], s2[:], -4.0, 8.0, op0=ALU.add, op1=ALU.mult)
            di = smp.tile([P, G], F32, tag="di")
            nc.gpsimd.tensor_sub(di[:], q1[:], kd[:])
            dn = smp.tile([P, G], F32, tag="dn")
            nc.gpsimd.tensor_scalar_max(dn[:], di[:], 1e-30)
            root = smp.tile([P, G], F32, tag="root")
            nc.scalar.activation(root[:], dn[:], ACTF.Sqrt)
            num = smp.tile([P, G], F32, tag="num")
            nc.gpsimd.tensor_sub(num[:], s1[:], root[:])
            th = smp.tile([P, G], F32, tag="th")
            nc.gpsimd.tensor_scalar_mul(th[:], num[:], 0.125)
            t_cur = smp.tile([P, G], F32, tag="t0")
            nc.gpsimd.tensor_add(t_cur[:], th[:], t8[:, :, 0])
            state[g]["t"] = t_cur

        def stage_newton(g, it):
            t_cur = state[g]["t"]
            last = it == n_newton - 1
            A1 = smp.tile([P, G], F32, tag="A1")
            A2 = smp.tile([P, G], F32, tag="A2")
            tm = smp.tile([P, G], F32, tag="tm")
            nc.gpsimd.tensor_scalar_mul(tm[:], t_cur[:], -1024.0)
            tb = smp.tile([P, G], F32, tag="tb")
            nc.gpsimd.tensor_scalar_mul(tb[:], t_cur[:], -0.5)
            mo_tiles = []
            for j in range(G):
                t_col = t_cur[:, j : j + 1]
                mo = (mol if last and not final_exact else mos).tile(
                    [P, D], F32, tag="mo"
                )
                nc.vector.tensor_scalar(
                    mo[:], xv(g, j), t_col, tm[:, j : j + 1],
                    op0=ALU.max, op1=ALU.add,
                    accum_out=A1[:, j : j + 1],
                )
                sq = atr.tile([P, D], F32, tag="at")
                nc.scalar.activation(
                    sq[:], mo[:], ACTF.Square, scale=0.5,
                    bias=tb[:, j : j + 1],
                    accum_out=A2[:, j : j + 1],
                )
                mo_tiles.append(mo)
            if last:
                state[g]["mo"] = mo_tiles
            # t += (A2 - 1) / max(A1/2, eps)
            s1h = smp.tile([P, G], F32, tag="s1h")
            nc.gpsimd.tensor_scalar(
                s1h[:], A1[:], 0.5, 1e-20, op0=ALU.mult, op1=ALU.max
            )
            rs = smp.tile([P, G], F32, tag="rs")
            nc.vector.reciprocal(rs[:], s1h[:])
            am = smp.tile([P, G], F32, tag="am")
            nc.gpsimd.tensor_scalar(am[:], A2[:], -1.0, None, op0=ALU.add)
            dtv = smp.tile([P, G], F32, tag="dtv")
            nc.gpsimd.tensor_mul(dtv[:], am[:], rs[:])
            t_new = smp.tile([P, G], F32, tag="tn")
            nc.gpsimd.tensor_add(t_new[:], dtv[:], t_cur[:])
            state[g]["t"] = t_new

        def stage_final(g):
            g_row0 = g * G * P
            t_cur = state[g]["t"]
            tb2 = smp.tile([P, G], F32, tag="tb2")
            nc.gpsimd.tensor_scalar_mul(tb2[:], t_cur[:], -0.5)
            nt2 = smp.tile([P, G], F32, tag="nt2")
            nc.gpsimd.tensor_scalar_mul(nt2[:], t_cur[:], -1.0)
            yt = None
            for j in range(G):
                if j % dma_batch == 0:
                    yt = yp.tile([P, dma_batch, D], F32, tag="y")
                mo = state[g]["mo"][j]
                if pool_final_mod and j % pool_final_mod == 0:
                    # Pool path: z = (mo + (-t2)) * 0.5; y = z*z (relieves
                    # ACT, which otherwise sits above the HBM roofline).
                    # Scalar operands follow the HW-proven (AP, const)
                    # slotting: scalar1 = per-partition AP, scalar2 = const.
                    zt = zp.tile([P, D], F32, tag="z")
                    nc.gpsimd.tensor_scalar(
                        zt[:], mo[:], nt2[:, j : j + 1], 0.5,
                        op0=ALU.add, op1=ALU.mult,
                    )
                    nc.gpsimd.tensor_mul(yt[:, j % dma_batch, :], zt[:], zt[:])
                else:
                    nc.scalar.activation(
                        yt[:, j % dma_batch, :], mo[:], ACTF.Square, scale=0.5,
                        bias=tb2[:, j : j + 1],
                    )
                if (j + 1) % dma_batch == 0:
                    r0 = g_row0 + (j + 1 - dma_batch) * P
                    if "no_dma_out" not in ablate:
                        nc.sync.dma_start(
                            y[r0 : r0 + dma_batch * P, :].rearrange(
                                "(a p) m -> p a m", p=P
                            ),
                            yt[:],
                        )
            state[g]["mo"] = None

        # pair-pipelined emission: two groups interleaved so each engine's
        # in-order stream has ready work while the other group's serial
        # t-update chain drains. repeats>1 re-emits the whole pass (same
        # input, same output) for steady-state throughput measurement by
        # R-differencing; every rep writes identical bytes to y.
        for _ in range(repeats):
            for p0 in range(0, n_groups, 2):
                pair = [g for g in (p0, p0 + 1) if g < n_groups]
                for g in pair:
                    stage_load_seed(g)
                for it in range(n_newton):
                    for g in pair:
                        stage_newton(g, it)
                for g in pair:
                    stage_final(g)

    nc.compile()
    return nc


_PROGRAM = None
_PROGRAM_ROWS = None


def _get_program(rows_per_core):
    global _PROGRAM, _PROGRAM_ROWS
    if _PROGRAM is None or _PROGRAM_ROWS != rows_per_core:
        _PROGRAM = build_program(rows_per_core)
        _PROGRAM_ROWS = rows_per_core
    return _PROGRAM


def run_sharded(flat_x, trace=False):
    """flat_x: [n_rows, 1024] fp32. Returns (y, BassKernelResults)."""
    from concourse.bass_utils import run_bass_kernel_spmd

    n_rows = flat_x.shape[0]
    rows_per = n_rows // N_CORES
    assert rows_per * N_CORES == n_rows
    nc = _get_program(rows_per)
    in_maps = [
        {"x": np.ascontiguousarray(flat_x[i * rows_per : (i + 1) * rows_per])}
        for i in range(N_CORES)
    ]
    res = run_bass_kernel_spmd(nc, in_maps, list(range(N_CORES)), trace=trace)
    y = np.concatenate([res.results[i]["y"] for i in range(N_CORES)], axis=0)
    return y, res


def kernel(x):
    x = np.ascontiguousarray(np.asarray(x), dtype=np.float32)
    orig_shape = x.shape
    flat = x.reshape(-1, D)
    y, _ = run_sharded(flat)
    return y.reshape(orig_shape)
